# revision 11
# baseline (speedup 1.0000x reference)
"""Causal multi-head attention (B=4, T=2048, D=1024, H=16) on 8 trn2 cores.

Sharding: core c -> (batch b = c//2, head-group g = c%2) -> 8 heads/core.
Per-core Bass kernel computes QKV projections, causal flash attention in
transposed-score orientation (s^T = K @ Q^T, softmax denominator via an
appended ones-column in V), and the head-sliced output projection partial.
Host sums the two head-group partials per batch (row-parallel proj).
"""

import numpy as np
import ml_dtypes

import concourse.bass as bass  # noqa: F401  (bass types via bacc)
import concourse.bacc as bacc
import concourse.mybir as mybir
import concourse.tile as tile
from concourse.bass_utils import run_bass_kernel_spmd

B, T, D = 4, 2048, 1024
H, DH = 16, 64
N_CORES = 8
HPC = 8      # heads per core
PAIRS = HPC // 2
BF = mybir.dt.bfloat16
F32 = mybir.dt.float32
BF_NP = ml_dtypes.bfloat16

TQ = 512     # q block (free dim)
TK = 128     # k block (partition dim)
NQG = T // TQ
NKC = T // TK


def build_nc():
    nc = bacc.Bacc(
        "TRN2",
        target_bir_lowering=False,
        debug=False,
        enable_asserts=True,
        num_devices=N_CORES,
    )
    xT = nc.dram_tensor("xT", [D, T], BF, kind="ExternalInput")
    wq = nc.dram_tensor("wq", [D, 512], BF, kind="ExternalInput")
    wk = nc.dram_tensor("wk", [D, 512], BF, kind="ExternalInput")
    wv = nc.dram_tensor("wv", [D, 512], BF, kind="ExternalInput")
    wp = nc.dram_tensor("wp", [512, D], BF, kind="ExternalInput")
    y = nc.dram_tensor("y", [T, D], F32, kind="ExternalOutput")

    with tile.TileContext(nc) as tc:
        with (
            tc.tile_pool(name="pers", bufs=1) as pers,
            tc.tile_pool(name="work", bufs=1) as work,
            tc.tile_pool(name="ps", bufs=1, space="PSUM") as pp,
        ):
            # ---- persistent SBUF ----
            xT_sb = pers.tile([128, 8, T], BF, tag="xT", name="xT_sb")
            wq_sb = pers.tile([128, 8, 512], BF, tag="wq", name="wq_sb")
            wk_sb = pers.tile([128, 8, 512], BF, tag="wk", name="wk_sb")
            wv_sb = pers.tile([128, 8, 512], BF, tag="wv", name="wv_sb")
            wp_sb = pers.tile([128, 4, D], BF, tag="wp", name="wp_sb")
            # V in token-major layout with a ones column per head: [tok, head, 65]
            vext = pers.tile([128, NKC, HPC, 65], BF, tag="vext", name="vext")
            # normalized attention outputs, d-major: [pair-chan, pair, tok]
            outT = pers.tile([128, PAIRS, T], BF, tag="outT", name="outT")
            # causal mask variants for diagonal blocks: keep q >= k + j*128
            mask_sb = pers.tile([128, 4, 512], BF, tag="mask", name="mask_sb")

            # ---- loads ----
            for dc in range(8):
                nc.sync.dma_start(xT_sb[:, dc, :], xT[dc * 128:(dc + 1) * 128, :])
                nc.sync.dma_start(wq_sb[:, dc, :], wq[dc * 128:(dc + 1) * 128, :])
                nc.sync.dma_start(wk_sb[:, dc, :], wk[dc * 128:(dc + 1) * 128, :])
                nc.sync.dma_start(wv_sb[:, dc, :], wv[dc * 128:(dc + 1) * 128, :])
            for cc in range(4):
                nc.sync.dma_start(wp_sb[:, cc, :], wp[cc * 128:(cc + 1) * 128, :])
            nc.gpsimd.memset(vext[:, :, :, 64], 1.0)
            nc.gpsimd.memset(mask_sb[:, :, :], 1.0)
            for j in range(4):
                nc.gpsimd.affine_select(
                    mask_sb[:, j, :],
                    mask_sb[:, j, :],
                    pattern=[[1, 512]],
                    compare_op=mybir.AluOpType.is_ge,
                    fill=0.0,
                    base=-j * TK,
                    channel_multiplier=-1,
                )

            # ---- phase 1: V = x @ wv  (token-major, all heads at once) ----
            for tk in range(NKC):
                ps_v = pp.tile([128, 512], F32, tag="acc", bufs=4, name="ps_v")
                for dc in range(8):
                    nc.tensor.matmul(
                        ps_v[:, :],
                        xT_sb[:, dc, tk * 128:(tk + 1) * 128],
                        wv_sb[:, dc, :],
                        start=(dc == 0),
                        stop=(dc == 7),
                    )
                nc.vector.tensor_copy(
                    vext[:, tk, :, 0:64],
                    ps_v.rearrange("p (h d) -> p h d", d=64),
                )

            # ---- phase 2: per head pair ----
            for hp in range(PAIRS):
                qt = work.tile([128, T], BF, tag="qt", bufs=2, name="qt")
                kt = work.tile([128, T], BF, tag="kt", bufs=2, name="kt")
                den = work.tile([128, 1024], F32, tag="den", bufs=2, name="den")
                nc.gpsimd.memset(den[:, :], 1.0)
                outU = {}
                # Q^T / K^T, d-major: rows = pair channels (head0: 0-63, head1: 64-127)
                for qg in range(NQG):
                    ps_q = pp.tile([128, 512], F32, tag="acc", bufs=4, name="ps_q")
                    ps_k = pp.tile([128, 512], F32, tag="acc", bufs=4, name="ps_k")
                    for dc in range(8):
                        nc.tensor.matmul(
                            ps_q[:, :],
                            wq_sb[:, dc, hp * 128:(hp + 1) * 128],
                            xT_sb[:, dc, qg * TQ:(qg + 1) * TQ],
                            start=(dc == 0),
                            stop=(dc == 7),
                        )
                    for dc in range(8):
                        nc.tensor.matmul(
                            ps_k[:, :],
                            wk_sb[:, dc, hp * 128:(hp + 1) * 128],
                            xT_sb[:, dc, qg * TQ:(qg + 1) * TQ],
                            start=(dc == 0),
                            stop=(dc == 7),
                        )
                    nc.scalar.copy(qt[:, qg * TQ:(qg + 1) * TQ], ps_q[:, :])
                    nc.scalar.copy(kt[:, qg * TQ:(qg + 1) * TQ], ps_k[:, :])

                # attention, software-pipelined: QK of chunk kc+1 is emitted
                # before AV of chunk kc so PE never waits on exp/mask
                for qg in range(NQG):
                    psO0 = pp.tile([65, 512], F32, tag="acc", bufs=4, name="psO0")
                    psO1 = pp.tile([65, 512], F32, tag="acc", bufs=4, name="psO1")
                    kmax = (qg + 1) * (TQ // TK)
                    noff = qg * (TQ // TK)

                    def qk(kc):
                        # scores^T chunk for both heads: [k 128, q 512] x2
                        # on diagonal blocks only columns q >= j*128 are live
                        off = max(0, kc - noff) * TK
                        ps_s = pp.tile([128, 1024], F32, tag="sc", bufs=2, name="ps_s")
                        for h in (0, 1):
                            nc.tensor.matmul(
                                ps_s[:, h * 512 + off:(h + 1) * 512],
                                kt[h * 64:(h + 1) * 64, kc * TK:(kc + 1) * TK],
                                qt[h * 64:(h + 1) * 64, qg * TQ + off:(qg + 1) * TQ],
                                start=True, stop=True,
                            )
                        return ps_s

                    def softmax_av(kc, ps_s):
                        off = max(0, kc - noff) * TK
                        j = kc - noff
                        ex = work.tile([128, 1024], BF, tag="ex", bufs=3, name="ex")
                        ex3 = ex.rearrange("p (h q) -> p h q", q=512)
                        ps_s3 = ps_s.rearrange("p (h q) -> p h q", q=512)
                        nc.scalar.activation(
                            ex3[:, :, off:512],
                            ps_s3[:, :, off:512],
                            mybir.ActivationFunctionType.Exp,
                        )
                        if j >= 0:
                            for h in (0, 1):
                                sl = slice(h * 512 + off, (h + 1) * 512)
                                nc.vector.tensor_mul(
                                    ex[:, sl], ex[:, sl], mask_sb[:, j, off:512]
                                )
                        for h, psO in ((0, psO0), (1, psO1)):
                            nc.tensor.matmul(
                                psO[:, off:512],
                                vext[:, kc, hp * 2 + h, :],
                                ex[:, h * 512 + off:(h + 1) * 512],
                                start=(kc == 0),
                                stop=(kc == kmax - 1),
                                skip_group_check=True,
                            )

                    prev = qk(0)
                    for kc in range(kmax):
                        nxt = qk(kc + 1) if kc + 1 < kmax else None
                        softmax_av(kc, prev)
                        prev = nxt

                    # evict unnormalized AV + denominator (row 64) to SBUF,
                    # freeing PSUM; stash denom rows into the pair's den tile
                    # (rows 32*qg, col-block h) for one batched reciprocal
                    for h, psO in ((0, psO0), (1, psO1)):
                        oU = work.tile([65, 512], F32, tag="outU", bufs=12,
                                       name="oU")
                        nc.vector.tensor_copy(oU[:, :], psO[:, :])
                        nc.vector.tensor_copy(
                            den[32 * qg:32 * qg + 1, h * 512:(h + 1) * 512],
                            psO[64:65, :],
                        )
                        outU[(qg, h)] = oU

                # batched reciprocal of all 8 denominators of this pair,
                # then broadcast + normalize into outT
                den_r = work.tile([128, 1024], F32, tag="denr", bufs=2, name="den_r")
                nc.vector.reciprocal(den_r[:, :], den[:, :])
                for qg in range(NQG):
                    for h in (0, 1):
                        # partition_broadcast only reads base partition 0 on
                        # HW, so stage the reciprocal row through partition 0
                        rc = work.tile([1, 512], F32, tag="rc", bufs=3, name="rc")
                        nc.vector.tensor_copy(
                            rc[0:1, :],
                            den_r[32 * qg:32 * qg + 1, h * 512:(h + 1) * 512],
                        )
                        bc = work.tile([64, 512], F32, tag="bc", bufs=3, name="bc")
                        nc.gpsimd.partition_broadcast(bc[0:64, :], rc[0:1, :])
                        nc.vector.tensor_mul(
                            outT[h * 64:(h + 1) * 64, hp, qg * TQ:(qg + 1) * TQ],
                            outU[(qg, h)][0:64, :],
                            bc[0:64, :],
                        )

            # ---- phase 3: y_partial = outT.T @ wp ----
            for tk in range(NKC):
                for nb in range(2):
                    ps_y = pp.tile([128, 512], F32, tag="acc", bufs=4, name="ps_y")
                    for cc in range(4):
                        nc.tensor.matmul(
                            ps_y[:, :],
                            outT[:, cc, tk * 128:(tk + 1) * 128],
                            wp_sb[:, cc, nb * 512:(nb + 1) * 512],
                            start=(cc == 0),
                            stop=(cc == 3),
                        )
                    y_ev = work.tile([128, 512], F32, tag="yev", bufs=3, name="y_ev")
                    nc.vector.tensor_copy(y_ev[:, :], ps_y[:, :])
                    nc.sync.dma_start(
                        y[tk * 128:(tk + 1) * 128, nb * 512:(nb + 1) * 512],
                        y_ev[:, :],
                    )

    nc.compile()
    return nc


_NC_CACHE = None


def _get_nc():
    global _NC_CACHE
    if _NC_CACHE is None:
        _NC_CACHE = build_nc()
    return _NC_CACHE


def make_in_maps(x, w_qkv, w_proj):
    """Host-side sharding: core c -> (batch c//2, head-group c%2)."""
    scale = np.float32(1.0 / np.sqrt(DH))
    in_maps = []
    for c in range(N_CORES):
        b, g = divmod(c, 2)
        sl = slice(g * 512, (g + 1) * 512)
        xT = np.ascontiguousarray(x[b].T).astype(BF_NP)
        wq = (w_qkv[:, 0 * D:1 * D][:, sl] * scale).astype(BF_NP)
        wk = w_qkv[:, 1 * D:2 * D][:, sl].astype(BF_NP)
        wv = w_qkv[:, 2 * D:3 * D][:, sl].astype(BF_NP)
        wp = np.ascontiguousarray(w_proj[sl, :]).astype(BF_NP)
        in_maps.append({"xT": xT, "wq": wq, "wk": wk, "wv": wv, "wp": wp})
    return in_maps


def kernel(x, w_qkv, w_proj, _trace=False, _tmpdir=None):
    x = np.asarray(x, dtype=np.float32)
    w_qkv = np.asarray(w_qkv, dtype=np.float32)
    w_proj = np.asarray(w_proj, dtype=np.float32)
    nc = _get_nc()
    in_maps = make_in_maps(x, w_qkv, w_proj)
    res = run_bass_kernel_spmd(
        nc, in_maps, core_ids=list(range(N_CORES)), trace=_trace, tmpdir=_tmpdir
    )
    out = np.empty((B, T, D), dtype=np.float32)
    for b in range(B):
        out[b] = res.results[2 * b]["y"] + res.results[2 * b + 1]["y"]
    if _trace:
        kernel._last_results = res
    return out


# revision 12
# speedup vs baseline: 1.0266x; 1.0266x over previous
"""Causal multi-head attention (B=4, T=2048, D=1024, H=16) on 8 trn2 cores.

Sharding: core c -> (batch b = c//2, head-group g = c%2) -> 8 heads/core.
Per-core Bass kernel computes QKV projections, causal flash attention in
transposed-score orientation (s^T = K @ Q^T, softmax denominator via an
appended ones-column in V), and the head-sliced output projection partial.
Host sums the two head-group partials per batch (row-parallel proj).
"""

import numpy as np
import ml_dtypes

import concourse.bass as bass  # noqa: F401  (bass types via bacc)
import concourse.bacc as bacc
import concourse.mybir as mybir
import concourse.tile as tile
from concourse.bass_utils import run_bass_kernel_spmd

B, T, D = 4, 2048, 1024
H, DH = 16, 64
N_CORES = 8
HPC = 8      # heads per core
PAIRS = HPC // 2
BF = mybir.dt.bfloat16
F32 = mybir.dt.float32
BF_NP = ml_dtypes.bfloat16

TQ = 512     # q block (free dim)
TK = 128     # k block (partition dim)
NQG = T // TQ
NKC = T // TK


def build_nc():
    nc = bacc.Bacc(
        "TRN2",
        target_bir_lowering=False,
        debug=False,
        enable_asserts=True,
        num_devices=N_CORES,
    )
    xT = nc.dram_tensor("xT", [D, T], BF, kind="ExternalInput")
    wq = nc.dram_tensor("wq", [D, 512], BF, kind="ExternalInput")
    wk = nc.dram_tensor("wk", [D, 512], BF, kind="ExternalInput")
    wv = nc.dram_tensor("wv", [D, 512], BF, kind="ExternalInput")
    wp = nc.dram_tensor("wp", [512, D], BF, kind="ExternalInput")
    y = nc.dram_tensor("y", [T, D], F32, kind="ExternalOutput")

    with tile.TileContext(nc) as tc:
        with (
            tc.tile_pool(name="pers", bufs=1) as pers,
            tc.tile_pool(name="work", bufs=1) as work,
            tc.tile_pool(name="ps", bufs=1, space="PSUM") as pp,
        ):
            # ---- persistent SBUF ----
            xT_sb = pers.tile([128, 8, T], BF, tag="xT", name="xT_sb")
            wq_sb = pers.tile([128, 8, 512], BF, tag="wq", name="wq_sb")
            wk_sb = pers.tile([128, 8, 512], BF, tag="wk", name="wk_sb")
            wv_sb = pers.tile([128, 8, 512], BF, tag="wv", name="wv_sb")
            wp_sb = pers.tile([128, 4, D], BF, tag="wp", name="wp_sb")
            # V in token-major layout with a ones column per head: [tok, head, 65]
            vext = pers.tile([128, NKC, HPC, 65], BF, tag="vext", name="vext")
            # normalized attention outputs, d-major: [pair-chan, pair, tok]
            outT = pers.tile([128, PAIRS, T], BF, tag="outT", name="outT")
            # causal mask variants for diagonal blocks: keep q >= k + j*128
            mask_sb = pers.tile([128, 128], BF, tag="mask", name="mask_sb")

            # ---- loads ----
            for dc in range(8):
                nc.sync.dma_start(xT_sb[:, dc, :], xT[dc * 128:(dc + 1) * 128, :])
                nc.sync.dma_start(wq_sb[:, dc, :], wq[dc * 128:(dc + 1) * 128, :])
                nc.sync.dma_start(wk_sb[:, dc, :], wk[dc * 128:(dc + 1) * 128, :])
                nc.sync.dma_start(wv_sb[:, dc, :], wv[dc * 128:(dc + 1) * 128, :])
            for cc in range(4):
                nc.sync.dma_start(wp_sb[:, cc, :], wp[cc * 128:(cc + 1) * 128, :])
            nc.gpsimd.memset(vext[:, :, :, 64], 1.0)
            nc.gpsimd.memset(mask_sb[:, :], 1.0)
            nc.gpsimd.affine_select(
                mask_sb[:, :],
                mask_sb[:, :],
                pattern=[[1, 128]],
                compare_op=mybir.AluOpType.is_ge,
                fill=0.0,
                base=0,
                channel_multiplier=-1,
            )

            # ---- phase 1: V = x @ wv  (token-major, all heads at once) ----
            for tk in range(NKC):
                ps_v = pp.tile([128, 512], F32, tag="accQ", bufs=2, name="ps_v")
                for dc in range(8):
                    nc.tensor.matmul(
                        ps_v[:, :],
                        xT_sb[:, dc, tk * 128:(tk + 1) * 128],
                        wv_sb[:, dc, :],
                        start=(dc == 0),
                        stop=(dc == 7),
                    )
                nc.vector.tensor_copy(
                    vext[:, tk, :, 0:64],
                    ps_v.rearrange("p (h d) -> p h d", d=64),
                )

            # ---- phase 2: per head pair ----
            for hp in range(PAIRS):
                qt = work.tile([128, T], BF, tag="qt", bufs=2, name="qt")
                kt = work.tile([128, T], BF, tag="kt", bufs=2, name="kt")
                den = work.tile([128, 1024], F32, tag="den", bufs=2, name="den")
                nc.gpsimd.memset(den[:, :], 1.0)
                outU = {}
                # Q^T / K^T, d-major: rows = pair channels (head0: 0-63, head1: 64-127)
                for qg in range(NQG):
                    ps_q = pp.tile([128, 512], F32, tag="accQ", bufs=2, name="ps_q")
                    ps_k = pp.tile([128, 512], F32, tag="accQ", bufs=2, name="ps_k")
                    for dc in range(8):
                        nc.tensor.matmul(
                            ps_q[:, :],
                            wq_sb[:, dc, hp * 128:(hp + 1) * 128],
                            xT_sb[:, dc, qg * TQ:(qg + 1) * TQ],
                            start=(dc == 0),
                            stop=(dc == 7),
                        )
                    for dc in range(8):
                        nc.tensor.matmul(
                            ps_k[:, :],
                            wk_sb[:, dc, hp * 128:(hp + 1) * 128],
                            xT_sb[:, dc, qg * TQ:(qg + 1) * TQ],
                            start=(dc == 0),
                            stop=(dc == 7),
                        )
                    nc.scalar.copy(qt[:, qg * TQ:(qg + 1) * TQ], ps_q[:, :])
                    nc.scalar.copy(kt[:, qg * TQ:(qg + 1) * TQ], ps_k[:, :])

                # attention, software-pipelined: QK of chunk kc+1 is emitted
                # before AV of chunk kc so PE never waits on exp/mask
                for qg in range(NQG):
                    psO0 = pp.tile([65, 512], F32, tag="accO", bufs=2, name="psO0")
                    psO1 = pp.tile([65, 512], F32, tag="accO", bufs=2, name="psO1")
                    kmax = (qg + 1) * (TQ // TK)
                    noff = qg * (TQ // TK)

                    def qk(kc):
                        # scores^T chunk for both heads: [k 128, q 512] x2
                        # on diagonal blocks only columns q >= j*128 are live
                        off = max(0, kc - noff) * TK
                        ps_s = pp.tile([128, 1024], F32, tag="sc", bufs=2, name="ps_s")
                        for h in (0, 1):
                            nc.tensor.matmul(
                                ps_s[:, h * 512 + off:(h + 1) * 512],
                                kt[h * 64:(h + 1) * 64, kc * TK:(kc + 1) * TK],
                                qt[h * 64:(h + 1) * 64, qg * TQ + off:(qg + 1) * TQ],
                                start=True, stop=True,
                            )
                        return ps_s

                    def softmax_av(kc, ps_s):
                        off = max(0, kc - noff) * TK
                        j = kc - noff
                        ex = work.tile([128, 1024], BF, tag="ex", bufs=4, name="ex")
                        ex3 = ex.rearrange("p (h q) -> p h q", q=512)
                        ps_s3 = ps_s.rearrange("p (h q) -> p h q", q=512)
                        nc.scalar.activation(
                            ex3[:, :, off:512],
                            ps_s3[:, :, off:512],
                            mybir.ActivationFunctionType.Exp,
                        )
                        if j >= 0:
                            for h in (0, 1):
                                sl = slice(h * 512 + off, h * 512 + off + TK)
                                nc.vector.tensor_mul(
                                    ex[:, sl], ex[:, sl], mask_sb[:, :]
                                )
                        for h, psO in ((0, psO0), (1, psO1)):
                            nc.tensor.matmul(
                                psO[:, off:512],
                                vext[:, kc, hp * 2 + h, :],
                                ex[:, h * 512 + off:(h + 1) * 512],
                                start=(kc == 0),
                                stop=(kc == kmax - 1),
                                skip_group_check=True,
                            )

                    prev = qk(0)
                    for kc in range(kmax):
                        nxt = qk(kc + 1) if kc + 1 < kmax else None
                        softmax_av(kc, prev)
                        prev = nxt

                    # evict unnormalized AV + denominator (row 64) to SBUF,
                    # freeing PSUM; stash denom rows into the pair's den tile
                    # (rows 32*qg, col-block h) for one batched reciprocal
                    for h, psO in ((0, psO0), (1, psO1)):
                        oU = work.tile([65, 512], F32, tag="outU", bufs=12,
                                       name="oU")
                        nc.vector.tensor_copy(oU[:, :], psO[:, :])
                        nc.vector.tensor_copy(
                            den[32 * qg:32 * qg + 1, h * 512:(h + 1) * 512],
                            psO[64:65, :],
                        )
                        outU[(qg, h)] = oU

                # batched reciprocal of all 8 denominators of this pair,
                # then broadcast + normalize into outT
                den_r = work.tile([128, 1024], F32, tag="denr", bufs=2, name="den_r")
                nc.vector.reciprocal(den_r[:, :], den[:, :])
                for qg in range(NQG):
                    for h in (0, 1):
                        # partition_broadcast only reads base partition 0 on
                        # HW, so stage the reciprocal row through partition 0
                        rc = work.tile([1, 512], F32, tag="rc", bufs=3, name="rc")
                        nc.vector.tensor_copy(
                            rc[0:1, :],
                            den_r[32 * qg:32 * qg + 1, h * 512:(h + 1) * 512],
                        )
                        bc = work.tile([64, 512], F32, tag="bc", bufs=3, name="bc")
                        nc.gpsimd.partition_broadcast(bc[0:64, :], rc[0:1, :])
                        nc.vector.tensor_mul(
                            outT[h * 64:(h + 1) * 64, hp, qg * TQ:(qg + 1) * TQ],
                            outU[(qg, h)][0:64, :],
                            bc[0:64, :],
                        )

            # ---- phase 3: y_partial = outT.T @ wp ----
            for tk in range(NKC):
                for nb in range(2):
                    ps_y = pp.tile([128, 512], F32, tag="accQ", bufs=2, name="ps_y")
                    for cc in range(4):
                        nc.tensor.matmul(
                            ps_y[:, :],
                            outT[:, cc, tk * 128:(tk + 1) * 128],
                            wp_sb[:, cc, nb * 512:(nb + 1) * 512],
                            start=(cc == 0),
                            stop=(cc == 3),
                        )
                    y_ev = work.tile([128, 512], F32, tag="yev", bufs=3, name="y_ev")
                    nc.scalar.copy(y_ev[:, :], ps_y[:, :])
                    nc.sync.dma_start(
                        y[tk * 128:(tk + 1) * 128, nb * 512:(nb + 1) * 512],
                        y_ev[:, :],
                    )

    nc.compile()
    return nc


_NC_CACHE = None


def _get_nc():
    global _NC_CACHE
    if _NC_CACHE is None:
        _NC_CACHE = build_nc()
    return _NC_CACHE


def make_in_maps(x, w_qkv, w_proj):
    """Host-side sharding: core c -> (batch c//2, head-group c%2)."""
    scale = np.float32(1.0 / np.sqrt(DH))
    in_maps = []
    for c in range(N_CORES):
        b, g = divmod(c, 2)
        sl = slice(g * 512, (g + 1) * 512)
        xT = np.ascontiguousarray(x[b].T).astype(BF_NP)
        wq = (w_qkv[:, 0 * D:1 * D][:, sl] * scale).astype(BF_NP)
        wk = w_qkv[:, 1 * D:2 * D][:, sl].astype(BF_NP)
        wv = w_qkv[:, 2 * D:3 * D][:, sl].astype(BF_NP)
        wp = np.ascontiguousarray(w_proj[sl, :]).astype(BF_NP)
        in_maps.append({"xT": xT, "wq": wq, "wk": wk, "wv": wv, "wp": wp})
    return in_maps


def kernel(x, w_qkv, w_proj, _trace=False, _tmpdir=None):
    x = np.asarray(x, dtype=np.float32)
    w_qkv = np.asarray(w_qkv, dtype=np.float32)
    w_proj = np.asarray(w_proj, dtype=np.float32)
    nc = _get_nc()
    in_maps = make_in_maps(x, w_qkv, w_proj)
    res = run_bass_kernel_spmd(
        nc, in_maps, core_ids=list(range(N_CORES)), trace=_trace, tmpdir=_tmpdir
    )
    out = np.empty((B, T, D), dtype=np.float32)
    for b in range(B):
        out[b] = res.results[2 * b]["y"] + res.results[2 * b + 1]["y"]
    if _trace:
        kernel._last_results = res
    return out
